# revision 30
# baseline (speedup 1.0000x reference)
"""Trainium2 Bass kernel for nn_CrossAttentionBlock (B=2, N=2048, C=1024, H=16).

Sharding: 8 cores; cores 0-3 handle batch 0, cores 4-7 batch 1. Within a batch
group each core owns a 512-token slice of the 2048 queries and computes the
K AND V projections for its own slice; per-chunk AllGathers share the
projected K^T / V so every core can attend its queries over the full batch.
The kt axis is stored core-locally-first (slot 0 = own slice, slots 1-3 =
rotated group peers) so attention can start on local tiles while the
collectives land.

On-core layout (bf16 compute, fp32 accumulation):
  qT/kT/vT   [c, tok]   transposed fp32->bf16 on the host
  qhT, khT   [hd, tok]  = W^T @ xT   (head h lives at partitions (h%2)*64)
  vhx        [tok-tile, head, 65] with a ones-column so the ctx matmul also
                        yields sum(exp)
  S^T        [kt, q]    = khT_h^T @ qhT_h
  softmax    exp only   (scores are O(1); max-subtraction provably unneeded)
  ctx^T      [65, q]    = vhx_h^T @ expS^T accumulated over kt in PSUM
  x          [tok, c]   = sum over head-pairs of ctxT^T @ Wo + q + bo
  FFN        y1T [ff, tok] = W1^T @ hT ; gelu ; y2 [tok, c] = gT^T @ W2
             (LayerNorm gamma/beta are folded into W1/b1 on the host)
"""
import sys

sys.path.insert(0, "/opt/trn_rl_repo")

import numpy as np
import ml_dtypes

import concourse.bass as bass
import concourse.tile as tile
from concourse import bacc, mybir
from concourse.bass_utils import run_bass_kernel_spmd
from concourse.masks import make_identity


def _ensure_ntff_hook():
    """The agent image's antenv package lacks axon_hooks; synthesize it so
    run_bass_kernel_spmd(trace=True) can reach the libaxon NTFF profiler."""
    import types
    if "antenv.axon_hooks" in sys.modules:
        return
    try:
        import antenv
    except ImportError:
        return
    mod = types.ModuleType("antenv.axon_hooks")
    mod._hook = None
    mod.set_axon_ntff_profile_hook = lambda h: setattr(mod, "_hook", h)
    mod.get_axon_ntff_profile_hook = lambda: mod._hook
    sys.modules["antenv.axon_hooks"] = mod
    antenv.axon_hooks = mod
    try:
        from trn_agent_boot.trn_boot import _ntff_profile_via_ctypes
        hook = _ntff_profile_via_ctypes("/opt/axon/libaxon_pjrt.so")
        if hook is not None:
            mod._hook = hook
    except Exception:
        pass


_ensure_ntff_hook()

P = 128
NT = 512          # q-tokens per core
KT = 2048         # keys per batch
B, N, C, H, HD, FF = 2, 2048, 1024, 16, 64, 2048
CT = C // P       # 8 c-tiles
JT = NT // P      # 4 tok-tiles per core
SCALE = HD ** -0.5

F32 = mybir.dt.float32
BF16 = mybir.dt.bfloat16
F8 = mybir.dt.float8e4
DR = mybir.MatmulPerfMode.DoubleRow
AF = mybir.ActivationFunctionType


def build():
    nc = bacc.Bacc(trn_type="TRN2")

    # ---- DRAM parameters (per-core shards; weights replicated) ----
    q_d = nc.declare_dram_parameter("q", [NT, C], F32, isOutput=False)
    kT_d = nc.declare_dram_parameter("kT", [P, CT, NT], F8, isOutput=False)
    vT_d = nc.declare_dram_parameter("vT", [KT // P, P, CT // 2, 2, P], F8, isOutput=False)
    qT_d = nc.declare_dram_parameter("qT", [P, CT, NT], F8, isOutput=False)
    Wk_d = nc.declare_dram_parameter("Wk", [CT, P, CT, P], F8, isOutput=False)
    Wv_d = nc.declare_dram_parameter("Wv", [P, CT // 2, 2, C], F8, isOutput=False)
    Wq_d = nc.declare_dram_parameter("Wq", [P, CT, C], F8, isOutput=False)
    Wo_d = nc.declare_dram_parameter("Wo", [P, CT, C], BF16, isOutput=False)
    W1a_d = nc.declare_dram_parameter("W1a", [P, CT // 2, FF], BF16, isOutput=False)
    W1b_d = nc.declare_dram_parameter("W1b", [P, CT // 2, FF], BF16, isOutput=False)
    W2a_d = nc.declare_dram_parameter("W2a", [P, CT, C], BF16, isOutput=False)
    W2b_d = nc.declare_dram_parameter("W2b", [P, CT, C], BF16, isOutput=False)
    bq_d = nc.declare_dram_parameter("bqt", [P, CT], F32, isOutput=False)
    bk_d = nc.declare_dram_parameter("bkt", [P, CT], F32, isOutput=False)
    b1_d = nc.declare_dram_parameter("b1t", [P, FF // P], F32, isOutput=False)
    bv_d = nc.declare_dram_parameter("bvb", [P, C], BF16, isOutput=False)
    bo_d = nc.declare_dram_parameter("bob", [P, C], BF16, isOutput=False)
    b2_d = nc.declare_dram_parameter("b2b", [P, C], BF16, isOutput=False)
    out_d = nc.declare_dram_parameter("out", [NT, C], F32, isOutput=True)

    with tile.TileContext(nc) as tc:
        with (
            tc.tile_pool(name="pers", bufs=1) as pers,
            tc.tile_pool(name="wpool", bufs=1) as wpool,
            tc.tile_pool(name="big", bufs=1) as big,
            tc.tile_pool(name="dram", bufs=1, space="DRAM") as dpool,
        ):
            # -------- priority DMAs: what the K projection needs first ------
            ident = pers.tile([P, P], BF16)
            make_identity(nc, ident[:])
            kT3 = big.tile([P, CT, NT], F8, tag="xT", bufs=2)
            for t in range(CT):
                nc.sync.dma_start(out=kT3[:, t, :], in_=kT_d[:, t, :])
            Wk_sb = wpool.tile([P, CT, CT, P], F8, tag="wB")
            for mm in range(CT):
                nc.scalar.dma_start(out=Wk_sb[:, mm], in_=Wk_d[mm])
            bq_sb = pers.tile([P, CT], F32)
            nc.scalar.dma_start(out=bq_sb[:], in_=bq_d[:])
            bk_sb = pers.tile([P, CT], F32)
            nc.scalar.dma_start(out=bk_sb[:], in_=bk_d[:])
            b1_sb = pers.tile([P, FF // P], F32)
            nc.scalar.dma_start(out=b1_sb[:], in_=b1_d[:])
            eps_sb = pers.tile([P, 1], F32)
            nc.vector.memset(eps_sb[:], 1e-5)

            # ---------------- persistent activations ----------------
            x_acc = big.tile([P, JT, C], F32)          # residual accumulator
            qhT3 = big.tile([P, CT, NT], BF16)         # [hd, q]
            khT3 = big.tile([P, CT, KT], F8, tag="Tkh")  # [hd, kt]; reused by gT3
            vhx = big.tile([P, KT // P, H, HD + 1], BF16, tag="Tvhx")  # hT3 later

            # K-only collective in fp8 (V is recomputed locally in fp8);
            # kt slots are rotated: slot 0 = own slice, slot s = group peer
            # (rloc+s)%4, matching the host-rotated vT tile order
            ag_in = dpool.tile([C, NT], F8, name="agin")
            ag_out = dpool.tile([8 * C, NT], F8, addr_space="Shared",
                                name="agout")
            RG8 = [[0, 1, 2, 3, 4, 5, 6, 7]]

            pid = nc.scalar.partition_id()
            g4 = (pid >> 2) * 4           # first core of this batch group
            rloc = pid - g4               # 0..3 slice index within the group

            # =========== PHASE A: load, project, all-gather ===========
            with (
                tc.tile_pool(name="ps", bufs=1, space="PSUM") as psA,
                tc.tile_pool(name="dynw", bufs=2) as stage,
            ):
                work = stage
                psB = psA
                psC = psA
                attw = stage
                ffw = stage

                # --- K projection of the local slice -> khT3 slot 0, AG ---
                for m in range(CT):
                    pk = psA.tile([P, NT], F32, tag="ctx", bufs=2)
                    for t2 in range(CT // 2):
                        nc.tensor.matmul(pk[:], Wk_sb[:, m, 2 * t2:2 * t2 + 2, :],
                                         kT3[:, 2 * t2:2 * t2 + 2, :],
                                         start=(t2 == 0), stop=(t2 == CT // 2 - 1),
                                         perf_mode=DR)
                    nc.vector.tensor_scalar_add(out=khT3[:, m, 0:NT], in0=pk[:],
                                                scalar1=bk_sb[:, m:m + 1])
                nc.scalar.dma_start(
                    out=ag_in[:].rearrange("(mm p) f -> p mm f", p=P),
                    in_=khT3[:, :, 0:NT])
                nc.gpsimd.collective_compute(
                    "AllGather", mybir.AluOpType.bypass,
                    ins=[ag_in[:]], outs=[ag_out[:]], replica_groups=RG8)
                # extract the 3 remote slices (slot s = group peer (rloc+s)%4)
                for s in range(1, 4):
                    rs = rloc + s
                    rot = rs - ((rs >> 2) * 4)
                    nc.scalar.dma_start(
                        out=khT3[:, :, s * NT:(s + 1) * NT],
                        in_=ag_out[bass.ds((g4 + rot) * C, C), :]
                        .rearrange("(mm p) f -> p mm f", p=P))

                # gate the next weight loads until Wk/kT3 have drained the
                # rings (the sequencers run far ahead of the data flow, so
                # without back-pressure these DMAs would bury the critical
                # K-projection loads and the collective staging)
                blkA = pers.tile([1, 8], F8)
                nc.gpsimd.tensor_copy(out=blkA[:], in_=Wk_sb[0:1, 7, 7, 120:128])
                Wv_sb = wpool.tile([P, CT // 2, 2, C], F8, tag="wC")
                nc.gpsimd.dma_start(out=Wv_sb[:], in_=Wv_d[:])
                bv_b = pers.tile([P, C], BF16)
                nc.gpsimd.dma_start(out=bv_b[:], in_=bv_d[:])
                Wq_sb = wpool.tile([P, CT, C], F8, tag="wA")
                nc.gpsimd.dma_start(out=Wq_sb[:], in_=Wq_d[:])
                qT3 = big.tile([P, CT, NT], F8, tag="xT", bufs=2)
                nc.gpsimd.dma_start(out=qT3[:], in_=qT_d[:])

                # --- V projection of the FULL batch, fp8 DoubleRow ---
                # (weights pre-scaled x64 on host; the 1/64 is folded into Wo
                # for the v-rows and cancels in the softmax normalization)
                nc.vector.memset(vhx[:, :, :, HD:HD + 1], 1.0)
                for i in range(KT // P):
                    vTc = stage.tile([P, CT // 2, 2, P], F8, tag="expS", bufs=5,
                                     name=f"vTc{i}")
                    nc.sync.dma_start(out=vTc[:], in_=vT_d[i])
                    pv = psA.tile([P, C], F32, tag="s", bufs=2, name=f"pv{i}")
                    for n in range(2):
                        for t2 in range(CT // 2):
                            nc.tensor.matmul(pv[:, n * NT:(n + 1) * NT],
                                             vTc[:, t2],
                                             Wv_sb[:, t2, :, n * NT:(n + 1) * NT],
                                             start=(t2 == 0), stop=(t2 == CT // 2 - 1),
                                             perf_mode=DR)
                    nc.vector.tensor_add(out=vhx[:, i, :, 0:HD],
                                         in0=pv[:].rearrange("p (h d) -> p h d", h=H),
                                         in1=bv_b[:].rearrange("p (h d) -> p h d", h=H))
                # --- Q projection ---
                for m in range(CT):
                    pq = psA.tile([P, NT], F32, tag="ctx", bufs=2)
                    for t2 in range(CT // 2):
                        nc.tensor.matmul(pq[:],
                                         Wq_sb[:, 2 * t2:2 * t2 + 2, m * P:(m + 1) * P],
                                         qT3[:, 2 * t2:2 * t2 + 2, :],
                                         start=(t2 == 0), stop=(t2 == CT // 2 - 1),
                                         perf_mode=DR)
                    nc.vector.tensor_scalar_add(out=qhT3[:, m, :], in0=pq[:],
                                                scalar1=bq_sb[:, m:m + 1])

                # deferred loads: blocked until mid-V-projection so the
                # collective staging + mesh get the DMA rings to themselves
                blkB = pers.tile([1, 8], BF16)
                nc.gpsimd.tensor_copy(out=blkB[:], in_=vhx[0:1, 11, 0, 0:8])
                Wo_sb = wpool.tile([P, CT, C], BF16, tag="wD")
                nc.gpsimd.dma_start(out=Wo_sb[:], in_=Wo_d[:])
                bo_b = pers.tile([P, C], BF16)
                nc.gpsimd.dma_start(out=bo_b[:], in_=bo_d[:])
                for j in range(JT):
                    raw = work.tile([P, C], F32, tag="bc", bufs=2, name=f"qraw{j}")
                    nc.gpsimd.dma_start(out=raw[:], in_=q_d[j * P:(j + 1) * P, :])
                    nc.vector.tensor_add(out=x_acc[:, j, :], in0=raw[:], in1=bo_b[:])
                W1a = wpool.tile([P, CT // 2, FF], BF16, tag="wA")
                nc.gpsimd.dma_start(out=W1a[:], in_=W1a_d[:])
                W1b = wpool.tile([P, CT // 2, FF], BF16, tag="wB")
                nc.gpsimd.dma_start(out=W1b[:], in_=W1b_d[:])
                W2a = wpool.tile([P, CT, C], BF16, tag="wC")
                nc.gpsimd.dma_start(out=W2a[:], in_=W2a_d[:])
                W2b = wpool.tile([P, CT, C], BF16, tag="wD")
                nc.gpsimd.dma_start(out=W2b[:], in_=W2b_d[:])
                b2_b = pers.tile([P, C], BF16)
                nc.gpsimd.dma_start(out=b2_b[:], in_=b2_d[:])

                # ======= PHASE B: attention + incremental out-projection =======
                stack = big.tile([P, CT, NT], BF16, tag="xT", bufs=2)  # ctx^T per pair
                KTO = list(range(16))
                mvs = ffw.tile([P, JT, 2], F32, tag="mvs", bufs=1)
                rstds = ffw.tile([P, JT], F32, tag="rstds", bufs=1)
                for pair in range(H // 2):
                    for parity in range(2):
                        h = 2 * pair + parity
                        p0 = parity * HD
                        ctx_ps = psB.tile([HD + 1, NT], F32, tag="ctx", bufs=2,
                                          name=f"ctx{pair}_{parity}")
                        for mega in range(8):
                            s_ps = psB.tile([P, 2, NT], F32, tag="s", bufs=2,
                                            name=f"s{pair}_{mega}_{parity}")
                            for jj in range(2):
                                i = KTO[mega * 2 + jj]
                                nc.tensor.matmul(
                                    s_ps[:, jj, :],
                                    khT3[p0:p0 + HD, pair, i * P:(i + 1) * P],
                                    qhT3[p0:p0 + HD, pair, :],
                                    start=True, stop=True)
                            expS = attw.tile([P, 2, NT], BF16, tag="expS", bufs=5,
                                             name=f"expS{pair}_{mega}_{parity}")
                            nc.scalar.activation(out=expS[:], in_=s_ps[:], func=AF.Exp,
                                                 scale=SCALE / 4096.0)
                            for jj in range(2):
                                i = KTO[mega * 2 + jj]
                                nc.tensor.matmul(
                                    ctx_ps[:], vhx[:, i, h, :], expS[:, jj, :],
                                    start=(mega == 0 and jj == 0),
                                    stop=(mega == 7 and jj == 1))
                        # normalize: rows 0..63 /= row 64
                        den_sb = attw.tile([1, NT], F32, tag="den", bufs=1,
                                           name=f"den{pair}_{parity}")
                        nc.vector.tensor_copy(out=den_sb[:], in_=ctx_ps[HD:HD + 1, :])
                        rc_sb = attw.tile([1, NT], F32, tag="rc", bufs=1,
                                          name=f"rc{pair}_{parity}")
                        nc.vector.reciprocal_approx_fast(out=rc_sb[:], in_=den_sb[:])
                        bc = attw.tile([HD, NT], F32, tag="bc", bufs=2,
                                       name=f"bc{pair}_{parity}")
                        nc.gpsimd.partition_broadcast(bc[:], rc_sb[0:1, :])
                        if parity == 0:
                            nc.vector.tensor_mul(out=stack[0:HD, pair, :],
                                                 in0=ctx_ps[0:HD, :], in1=bc[:])
                        else:
                            todd = attw.tile([HD, NT], BF16, tag="todd", bufs=2,
                                             name=f"todd{pair}")
                            nc.vector.tensor_mul(out=todd[:], in0=ctx_ps[0:HD, :], in1=bc[:])
                            nc.sync.dma_start(out=stack[HD:P, pair, :], in_=todd[:])
                    # incremental out-projection for this head pair
                    for j in range(JT):
                        for n in range(2):
                            op = psB.tile([P, NT], F32, tag="op", bufs=2)
                            nc.tensor.matmul(op[:], stack[:, pair, j * P:(j + 1) * P],
                                             Wo_sb[:, pair, n * NT:(n + 1) * NT],
                                             start=True, stop=True)
                            nc.vector.tensor_add(out=x_acc[:, j, n * NT:(n + 1) * NT],
                                                 in0=x_acc[:, j, n * NT:(n + 1) * NT],
                                                 in1=op[:])
                        if pair == H // 2 - 1:
                            # LN stats as soon as this token tile is final
                            st = ffw.tile([P, 2, 6], F32, tag="st", bufs=2,
                                          name=f"st{j}")
                            for sgt in range(2):
                                nc.vector.bn_stats(out=st[:, sgt, :],
                                                   in_=x_acc[:, j, sgt * NT:(sgt + 1) * NT])
                            nc.vector.bn_aggr(out=mvs[:, j, :], in_=st[:])
                            nc.scalar.activation(out=rstds[:, j:j + 1], in_=mvs[:, j, 1:2],
                                                 func=AF.Sqrt, bias=eps_sb[:])
                            nc.vector.reciprocal(out=rstds[:, j:j + 1],
                                                 in_=rstds[:, j:j + 1])

                # ======= PHASE C: LayerNorm (folded), FFN, residual, store ====
                hT3 = big.tile([P, CT, NT], BF16, tag="Tvhx")
                for j in range(JT):
                    hj = ffw.tile([P, C], BF16, tag="expS", bufs=5, name=f"hj{j}")
                    nc.vector.tensor_scalar(out=hj[:], in0=x_acc[:, j, :],
                                            scalar1=mvs[:, j, 0:1], scalar2=rstds[:, j:j + 1],
                                            op0=mybir.AluOpType.subtract,
                                            op1=mybir.AluOpType.mult)
                    for t in range(CT):
                        tp = psC.tile([P, P], BF16, tag="op", bufs=2, name=f"htp{j}_{t}")
                        nc.tensor.transpose(tp[:], hj[:, t * P:(t + 1) * P], ident[:])
                        nc.vector.tensor_copy(out=hT3[:, t, j * P:(j + 1) * P], in_=tp[:])

                gT3 = big.tile([P, FF // P, NT], BF16, tag="Tkh")
                for mf in range(FF // P):
                    pf = psC.tile([P, NT], F32, tag="s", bufs=2)
                    for t in range(CT):
                        wsl = W1a[:, t, mf * P:(mf + 1) * P] if t < 4 else \
                            W1b[:, t - 4, mf * P:(mf + 1) * P]
                        nc.tensor.matmul(pf[:], wsl, hT3[:, t, :],
                                         start=(t == 0), stop=(t == CT - 1))
                    nc.scalar.activation(out=gT3[:, mf, :], in_=pf[:], func=AF.Gelu,
                                         bias=b1_sb[:, mf:mf + 1])

                for j in range(JT):
                    xb = ffw.tile([P, C], F32, tag="bc", bufs=2)
                    nc.vector.tensor_add(out=xb[:], in0=x_acc[:, j, :], in1=b2_b[:])
                    out_sb = ffw.tile([P, C], F32, tag="bc", bufs=2)
                    for n in range(2):
                        pf2 = psC.tile([P, NT], F32, tag="ctx", bufs=2)
                        for t2 in range(FF // P):
                            w2sl = W2a[:, t2, n * NT:(n + 1) * NT] if t2 < CT else \
                                W2b[:, t2 - CT, n * NT:(n + 1) * NT]
                            nc.tensor.matmul(pf2[:], gT3[:, t2, j * P:(j + 1) * P], w2sl,
                                             start=(t2 == 0), stop=(t2 == FF // P - 1))
                        nc.vector.tensor_add(out=out_sb[:, n * NT:(n + 1) * NT],
                                             in0=pf2[:], in1=xb[:, n * NT:(n + 1) * NT])
                    nc.sync.dma_start(out=out_d[j * P:(j + 1) * P, :], in_=out_sb[:])

    nc.compile()
    return nc


_NC = None
LAST_RESULT = None


def kernel(q, k, v, Wq, bq, Wk, bk, Wv, bv, Wo, bo, ln_w, ln_b, W1, b1, W2, b2):
    global _NC, LAST_RESULT
    if _NC is None:
        _NC = build()
    bf = ml_dtypes.bfloat16

    # fold the post-attention LayerNorm affine into the first FFN matmul
    W1f = (np.asarray(ln_w, np.float32)[:, None] * np.asarray(W1, np.float32))
    b1f = (np.asarray(b1, np.float32)
           + np.asarray(ln_b, np.float32) @ np.asarray(W1, np.float32))

    def wlay(w, rows=None, dt=None):
        dt = dt or bf
        w = np.asarray(w, dtype=dt) if rows is None else np.asarray(w[rows[0]:rows[1]], dtype=dt)
        r, c = w.shape
        return np.ascontiguousarray(w.reshape(r // P, P, c).transpose(1, 0, 2))

    f8 = ml_dtypes.float8_e4m3
    # V path runs in fp8 DoubleRow with weights pre-scaled by 64 (the scale
    # cancels against 1/64 folded into Wo; the softmax denominator is
    # scale-invariant). Contraction channel c pairs as (p, i) = (c//2, c%2).
    Wv8 = (64.0 * np.asarray(Wv, np.float32)).reshape(CT // 2, 2, P, C)
    Wv8 = np.ascontiguousarray(Wv8.transpose(2, 0, 1, 3).astype(f8))
    shared = {
        "Wq": wlay(64.0 * np.asarray(Wq, np.float32), dt=f8), "Wv": Wv8,
        "Wo": wlay(np.asarray(Wo, np.float32) / 64.0),
        "Wk": np.ascontiguousarray(
            (64.0 * np.asarray(Wk, np.float32)).astype(f8)
            .reshape(CT, P, CT, P).transpose(2, 1, 0, 3)),
        "W1a": wlay(W1f, (0, C // 2)), "W1b": wlay(W1f, (C // 2, C)),
        "W2a": wlay(W2, (0, C)), "W2b": wlay(W2, (C, FF)),
        "bqt": np.ascontiguousarray((64.0 * np.asarray(bq, np.float32)).reshape(CT, P).T),
        "bkt": np.ascontiguousarray((64.0 * np.asarray(bk, np.float32)).reshape(CT, P).T),
        "b1t": np.ascontiguousarray(np.asarray(b1f, np.float32).reshape(FF // P, P).T),
        "bvb": np.ascontiguousarray(
            np.broadcast_to((64.0 * np.asarray(bv, np.float32)).astype(bf), (P, C))),
        "bob": np.ascontiguousarray(np.broadcast_to(np.asarray(bo, bf), (P, C))),
        "b2b": np.ascontiguousarray(np.broadcast_to(np.asarray(b2, bf), (P, C))),
    }
    in_maps = []
    for i in range(8):
        b, r = i // 4, i % 4
        m = dict(shared)
        m["q"] = np.ascontiguousarray(q[b, r * NT:(r + 1) * NT], np.float32)
        for nm, full in (("qT", q), ("kT", k)):
            sh = np.asarray(full[b, r * NT:(r + 1) * NT], np.float32).T.astype(f8)
            m[nm] = np.ascontiguousarray(sh.reshape(CT, P, NT).transpose(1, 0, 2))
        vsh = np.asarray(v[b], np.float32).T.astype(f8)      # [C, KT] full batch
        vtiles = vsh.reshape(CT // 2, 2, P, KT // P, P).transpose(3, 2, 0, 1, 4)
        # kt-tile rotation: slot s holds group peer (r+s)%4's slice, matching
        # the rotated khT3 extraction
        order = [((r + s) % 4) * 4 + i for s in range(4) for i in range(4)]
        m["vT"] = np.ascontiguousarray(vtiles[order])
        in_maps.append(m)
    LAST_RESULT = run_bass_kernel_spmd(_NC, in_maps, core_ids=list(range(8)))
    out = np.empty((B, N, C), np.float32)
    for i in range(8):
        b, r = i // 4, i % 4
        out[b, r * NT:(r + 1) * NT] = LAST_RESULT.results[i]["out"]
    return out
